# revision 10
# baseline (speedup 1.0000x reference)
"""AdaptiveDilatedConv2d on 8 TRN2 NeuronCores.

Factorization (validated vs reference in numpy):
  out[o,r,s] = bias[o] + sum_t sum_{dy,dx} Wr_t[dy](r,s)*Wc_t[dx](r,s)*Z_t[o,r+dy,s+dx]
  Z_t[o,q] = sum_c weight[o,c,t] * x[c,q]
Stage 1 (PE): Z_t^T[w,o] per image row via matmul (lhsT = x row slice).
Stage 2 (PE): output rows processed in blocks of B=4. For each (tap t,
input row g) the matmuls serving the rows r = g - dy of the block are
fused into ONE wide matmul (N = 126 * n_rows <= 504) whose rhs is a
host-packed mask tile and whose out spans adjacent PSUM slots, so each
Z-tile weight load serves up to 4 output rows (LDWEIGHTS amortized).

Sharding: core k handles images (2*(k//2), 2*(k//2)+1), output rows
[63*(k%2), 63*(k%2)+63). Mask tiles are shared by both images on a core.
"""
import numpy as np
import ml_dtypes

import concourse.bass as bass
import concourse.mybir as mybir
import concourse.tile as tile
from concourse import bacc
from concourse.bass import ts
from concourse.bass_utils import run_bass_kernel_spmd

K = 3
C = 128           # in channels
O = 128           # out channels
H = W = 128
Ho = Wo = 126
NIMG = 8
DYMAX = 7         # dy/dx in 0..6
ROWS_HALF = 63    # output rows per core
SLAB = ROWS_HALF + DYMAX - 1  # 69 input rows per core per image (h = 0..68)
NT = K * K        # 9 taps
WPACK = NT * O    # 1152
B = 4             # output rows per stage-2 block


def _dkh(k):
    return [0] if k == 0 else ([1, 2, 3] if k == 1 else [2, 3, 4, 5, 6])


# fixed (tap, dy) enumeration shared by host packing and device program.
# Tap (0,0) at dy=0 is identity sampling (mask == I): handled as a direct
# matmul from the x row with W00 stationary, so it is excluded here.
PAIRS = [(kh * K + kw, kh, kw, dy)
         for kh in range(K) for kw in range(K) for dy in _dkh(kh)][1:]
NPAIR = len(PAIRS)  # 26
PAIR_IDX = {(t, dy): i for i, (t, kh, kw, dy) in enumerate(PAIRS)}

# two small leading blocks so stage-2 work starts at h=7 instead of h=9,
# filling the PE during the stage-1-only ramp; then 4-row blocks.
BLOCKS = [(0, 2), (2, 2)] + \
    [(u0, min(B, ROWS_HALF - u0)) for u0 in range(4, ROWS_HALF, B)]
NBLK = len(BLOCKS)  # 17


def _tap_support(t):
    kh = t // K
    return (0, 0) if kh == 0 else ((1, 3) if kh == 1 else (2, 6))


def _block_mms(u0, nb):
    """Static list of fused stage-2 matmuls for one block.

    Returns ([(t, g, rlo, rhi, col_off)], total_cols): tap t's Z row g
    serves output rows rlo..rhi of the block; mask cols at col_off.
    """
    out = []
    off = 0
    for t in range(1, NT):
        d0, d1 = _tap_support(t)
        for g in range(u0 + d0, u0 + nb - 1 + d1 + 1):
            rlo = max(u0, g - d1)
            rhi = min(u0 + nb - 1, g - d0)
            if rlo > rhi:
                continue
            out.append((t, g, rlo, rhi, off))
            off += (rhi - rlo + 1) * Wo
    return out, off


BLKCOLS = B * NPAIR * Wo  # 13104 (full block); last block uses a prefix


def _interp_bilinear(r, out_h, out_w):
    in_h, in_w = r.shape

    def src(n_out, n_in):
        s = (np.arange(n_out, dtype=np.float32) + 0.5) * (n_in / n_out) - 0.5
        return np.clip(s, 0.0, n_in - 1.0)

    sy = src(out_h, in_h)
    sx = src(out_w, in_w)
    y0 = np.floor(sy).astype(np.int32)
    x0 = np.floor(sx).astype(np.int32)
    y1 = np.minimum(y0 + 1, in_h - 1)
    x1 = np.minimum(x0 + 1, in_w - 1)
    wy = (sy - y0)[:, None]
    wx = (sx - x0)[None, :]
    return (r[y0[:, None], x0[None, :]] * (1 - wy) * (1 - wx)
            + r[y0[:, None], x1[None, :]] * (1 - wy) * wx
            + r[y1[:, None], x0[None, :]] * wy * (1 - wx)
            + r[y1[:, None], x1[None, :]] * wy * wx)


def _build_mask_arrays(rates):
    """Wr[k, d, r, s], Wc[k, d, r, s] float32 with OOB zeroing."""
    rate = _interp_bilinear(rates[0, 0].astype(np.float32), Ho, Wo)
    Wr = np.zeros((K, DYMAX, Ho, Wo), np.float32)
    Wc = np.zeros((K, DYMAX, Ho, Wo), np.float32)
    rr = np.arange(Ho)[:, None]
    ss = np.arange(Wo)[None, :]
    for k in range(K):
        u = k * rate
        f = np.floor(u).astype(np.int32)
        w = u - f
        for d in range(DYMAX):
            v = (f == d) * (1 - w) + (f + 1 == d) * w
            Wr[k, d] = v * (rr + d < H)
            Wc[k, d] = v * (ss + d < W)
    return Wr, Wc


def _build_row_masks(rates, r0):
    """[ROWS_HALF, W, NPAIR*Wo] f32 banded per-row mask tiles (rows r0+...)."""
    Wr, Wc = _build_mask_arrays(rates)
    out = np.zeros((ROWS_HALF, W, NPAIR * Wo), np.float32)
    s = np.arange(Wo)
    for i, (t, kh, kw, dy) in enumerate(PAIRS):
        for dx in range(DYMAX):
            q = s + dx
            valid = q < W
            sv = s[valid]
            M = (Wr[kh, dy, r0:r0 + ROWS_HALF, :][:, sv]
                 * Wc[kw, dx, r0:r0 + ROWS_HALF, :][:, sv])
            rl = np.arange(ROWS_HALF)[:, None]
            out[rl, q[valid][None, :], (i * Wo + sv)[None, :]] = M
    return out


def _pack_block_masks(row_masks):
    """[NBLK, W, BLKCOLS] bf16: per-block fused-matmul rhs tiles."""
    out = np.zeros((NBLK, W, BLKCOLS), np.float32)
    for k, (u0, nb) in enumerate(BLOCKS):
        mms, total = _block_mms(u0, nb)
        assert total == nb * NPAIR * Wo
        for (t, g, rlo, rhi, off) in mms:
            for j, r in enumerate(range(rlo, rhi + 1)):
                i = PAIR_IDX[(t, g - r)]
                out[k, :, off + j * Wo: off + (j + 1) * Wo] = \
                    row_masks[r][:, i * Wo:(i + 1) * Wo]
    return out.astype(ml_dtypes.bfloat16)


def build_nc(repeat=1):
    """Build the SPMD program (same for every core)."""
    nc = bacc.Bacc("TRN2", target_bir_lowering=False, debug=False, num_devices=8)
    bf16 = mybir.dt.bfloat16
    f32 = mybir.dt.float32

    x_d = nc.dram_tensor("x", [2, C, SLAB, W], bf16, kind="ExternalInput")
    w_d = nc.dram_tensor("wpack", [C, WPACK], bf16, kind="ExternalInput")
    b_d = nc.dram_tensor("bias", [O, 1], f32, kind="ExternalInput")
    m_d = nc.dram_tensor("masks", [NBLK, W, BLKCOLS], bf16, kind="ExternalInput")
    o_d = nc.dram_tensor("out", [2, O, ROWS_HALF, Wo], f32, kind="ExternalOutput")

    with tile.TileContext(nc) as tc:
        with (
            tc.tile_pool(name="xp", bufs=4) as xp,
            tc.tile_pool(name="wp", bufs=1) as wp,
            tc.tile_pool(name="zp", bufs=22) as zp,
            tc.tile_pool(name="mp", bufs=3) as mp,
            tc.tile_pool(name="op", bufs=4) as op,
            tc.tile_pool(name="ps1", bufs=3, space="PSUM") as ps1,
            tc.tile_pool(name="acc", bufs=5, space="PSUM") as accp,
        ):
            wt = wp.tile([C, WPACK], bf16, tag="w")
            nc.sync.dma_start(out=wt[:, :], in_=w_d[:, :])
            bt = wp.tile([O, 1], f32, tag="b")
            nc.sync.dma_start(out=bt[:, :], in_=b_d[:, :])

            def body(it):
                xts = []
                for img in range(2):
                    xt = xp.tile([C, SLAB, W], bf16, tag=f"x{img}")
                    # head chunks land quickly so stage-1 can start early
                    nc.sync.dma_start(out=xt[:, :2, :], in_=x_d[img][:, :2, :])
                    nc.sync.dma_start(out=xt[:, 2:8, :], in_=x_d[img][:, 2:8, :])
                    xts.append(xt)

                zrows = [{}, {}]  # img -> h -> tile
                mts = {}
                # x rows 8-24 land before the mask prefetch (stage-1 h<=23
                # must not wait behind 6.7MB of masks); masks for blocks 0-1
                # land before the x tail (first fire is at h=9).
                for img in range(2):
                    nc.sync.dma_start(out=xts[img][:, 8:24, :],
                                      in_=x_d[img][:, 8:24, :])
                for k in range(2):
                    mt = mp.tile([W, BLKCOLS], bf16, tag="m", name=f"mt{k}")
                    nc.sync.dma_start(out=mt[:, :], in_=m_d[k])
                    mts[k] = mt
                for img in range(2):
                    nc.sync.dma_start(out=xts[img][:, 24:, :],
                                      in_=x_d[img][:, 24:, :])

                # stage-1 weight chunks by h validity:
                #   h<=62: taps 1-8, 63<=h<=65: taps 3-8, 66<=h<=68: taps 6-8
                def stage1(h, img):
                    zt = zp.tile([W, WPACK - O], bf16, tag="z")
                    if h <= ROWS_HALF - 1:
                        chunks = [(0, 512), (512, 512)]
                    elif h <= ROWS_HALF + 2:
                        chunks = [(256, 384), (640, 384)]
                    else:
                        chunks = [(640, 384)]
                    for ci, (z0, width) in enumerate(chunks):
                        p1 = ps1.tile([W, 512], f32, tag="s1")
                        nc.tensor.matmul(
                            p1[:, :width],
                            xts[img][:, h, :],
                            wt[:, O + z0:O + z0 + width],
                            start=True, stop=True,
                        )
                        eng = nc.vector.tensor_copy if (ci + h) % 2 == 0 \
                            else nc.scalar.copy
                        eng(zt[:, z0:z0 + width], p1[:, :width])
                    zrows[img][h] = zt

                def fire_block(k):
                    u0, nb = BLOCKS[k]
                    mms, total = _block_mms(u0, nb)
                    mt = mts.pop(k)
                    for img in range(2):
                        acc = accp.tile([O, B * Wo], f32, tag="acc")
                        # start=True pending-zeroes the whole 2KB PSUM bank
                        # (zero-region granularity): one fused identity matmul
                        # opens the block's accumulation; everything after
                        # accumulates onto pending-zero.
                        nc.tensor.matmul(
                            acc[:, :nb * Wo],
                            wt[:, :O],
                            xts[img][:, u0:u0 + nb, :Wo],
                            start=True, stop=False,
                        )
                        for mi, (t, g, rlo, rhi, off) in enumerate(mms):
                            n = (rhi - rlo + 1) * Wo
                            nc.tensor.matmul(
                                acc[:, (rlo - u0) * Wo:(rhi + 1 - u0) * Wo],
                                zrows[img][g][:, ts(t - 1, O)],
                                mt[:, off:off + n],
                                start=False, stop=(mi == len(mms) - 1),
                            )
                        ost = op.tile([O, B, Wo], f32, tag="o")
                        nc.scalar.activation(
                            ost[:, :nb, :], acc[:, :nb * Wo],
                            mybir.ActivationFunctionType.Identity,
                            bias=bt[:, :], scale=1.0,
                        )
                        # output DMA on the Activation HWDGE queue so it is
                        # not serialized behind the mask stream on SP's queue
                        nc.scalar.dma_start(
                            out=o_d[img][:, u0:u0 + nb, :],
                            in_=ost[:, :nb, :])

                # block k fires after stage-1 of row 4k+9 (needs Z up to
                # 4k+9); its masks are prefetched ~8 rows earlier.
                fire_at = {}
                for k, (u0, nb) in enumerate(BLOCKS):
                    fire_at[min(u0 + nb - 1 + DYMAX - 1, SLAB - 1)] = k
                for h in range(SLAB):
                    k_dma = (h - 1) // B + 2
                    if h >= 1 and (h - 1) % B == 0 and k_dma < NBLK:
                        mt = mp.tile([W, BLKCOLS], bf16, tag="m",
                                     name=f"mt{k_dma}")
                        nc.sync.dma_start(out=mt[:, :], in_=m_d[k_dma])
                        mts[k_dma] = mt
                    for img in range(2):
                        stage1(h, img)
                    if h in fire_at:
                        fire_block(fire_at[h])

            if repeat == 1:
                body(0)
            else:
                with tc.For_i(0, repeat, 1):
                    body(0)

    nc.compile()
    return nc


def _prep_core_inputs(inputs, weight, rates, bias):
    """Returns list of 8 in_maps (host-side shard + mask precompute)."""
    x = np.asarray(inputs)
    wgt = np.asarray(weight)
    b = np.asarray(bias)

    # wpack[c, t*O + o] = weight[o, c, kh, kw],  t = kh*K + kw
    wpack = np.transpose(wgt.reshape(O, C, NT), (1, 2, 0)).reshape(C, NT * O)
    wpack = np.ascontiguousarray(wpack).astype(ml_dtypes.bfloat16)
    b2 = np.ascontiguousarray(b.reshape(O, 1)).astype(np.float32)

    masks_by_half = [
        _pack_block_masks(_build_row_masks(np.asarray(rates), 0)),
        _pack_block_masks(_build_row_masks(np.asarray(rates), ROWS_HALF)),
    ]

    in_maps = []
    for k in range(8):
        a = 2 * (k // 2)
        half = k % 2
        r0 = ROWS_HALF * half
        slab = np.zeros((2, C, SLAB, W), np.float32)
        avail = min(SLAB, H - r0)
        slab[0, :, :avail, :] = x[a, :, r0:r0 + avail, :]
        slab[1, :, :avail, :] = x[a + 1, :, r0:r0 + avail, :]
        in_maps.append({
            "x": slab.astype(ml_dtypes.bfloat16),
            "wpack": wpack,
            "bias": b2,
            "masks": masks_by_half[half],
        })
    return in_maps


_NC_CACHE = {}


def _get_nc(repeat=1):
    if repeat not in _NC_CACHE:
        _NC_CACHE[repeat] = build_nc(repeat)
    return _NC_CACHE[repeat]


def kernel(inputs, weight, rates, bias):
    nc = _get_nc(1)
    in_maps = _prep_core_inputs(inputs, weight, rates, bias)
    res = run_bass_kernel_spmd(nc, in_maps, core_ids=list(range(8)))
    out = np.zeros((NIMG, O, Ho, Wo), np.float32)
    for k in range(8):
        a = 2 * (k // 2)
        half = k % 2
        r0 = ROWS_HALF * half
        o = res.results[k]["out"]
        out[a, :, r0:r0 + ROWS_HALF, :] = o[0]
        out[a + 1, :, r0:r0 + ROWS_HALF, :] = o[1]
    return out


# revision 14
# speedup vs baseline: 1.0821x; 1.0821x over previous
"""AdaptiveDilatedConv2d on 8 TRN2 NeuronCores.

Factorization (validated vs reference in numpy):
  out[o,r,s] = bias[o] + sum_t sum_{dy,dx} Wr_t[dy](r,s)*Wc_t[dx](r,s)*Z_t[o,r+dy,s+dx]
  Z_t[o,q] = sum_c weight[o,c,t] * x[c,q]
Stage 1 (PE): Z_t^T[w,o] per image row via matmul (lhsT = x row slice).
Stage 2 (PE): output rows processed in blocks (2,2,4,4,...). For each
(tap t, input row g) the matmuls serving rows r = g - dy of the block
are fused into ONE wide matmul (N = 126 * n_rows <= 504) whose rhs is a
host-packed mask slice and whose out spans adjacent PSUM slots, so each
Z-tile weight load serves up to 4 output rows (LDWEIGHTS amortized).

The fused matmuls consume mask columns strictly sequentially, so masks
live in DRAM as ONE flat [128, 206388] bf16 tensor streamed through a
small ring of fixed-size SBUF chunk tiles on the SP DMA queue; x and
output transfers ride the Activation DMA queue.

Sharding: core k handles images (2*(k//2), 2*(k//2)+1), output rows
[63*(k%2), 63*(k%2)+63). Mask tiles are shared by both images on a core.
"""
import numpy as np
import ml_dtypes

import concourse.bass as bass
import concourse.mybir as mybir
import concourse.tile as tile
from concourse import bacc
from concourse.bass import ts
from concourse.bass_utils import run_bass_kernel_spmd

K = 3
C = 128           # in channels
O = 128           # out channels
H = W = 128
Ho = Wo = 126
NIMG = 8
DYMAX = 7         # dy/dx in 0..6
ROWS_HALF = 63    # output rows per core
SLAB = ROWS_HALF + DYMAX - 1  # 69 input rows per core per image (h = 0..68)
XHEAD = 8         # x rows 0..7 in their own (cross-iteration) pool
NT = K * K        # 9 taps
WPACK = NT * O    # 1152
B = 4             # output rows per full stage-2 block


def _dkh(k):
    return [0] if k == 0 else ([1, 2, 3] if k == 1 else [2, 3, 4, 5, 6])


# fixed (tap, dy) enumeration shared by host packing and device program.
# Tap (0,0) at dy=0 is identity sampling (mask == I): handled as a direct
# matmul from the x row with W00 stationary, so it is excluded here.
PAIRS = [(kh * K + kw, kh, kw, dy)
         for kh in range(K) for kw in range(K) for dy in _dkh(kh)][1:]
NPAIR = len(PAIRS)  # 26
PAIR_IDX = {(t, dy): i for i, (t, kh, kw, dy) in enumerate(PAIRS)}

# two small leading blocks so stage-2 work starts at h=7 instead of h=9,
# filling the PE during the stage-1-only ramp; then 4-row blocks. No block
# straddles the XHEAD boundary (rows 0-1, 2-3, 4-7, 8-11, ...).
BLOCKS = [(0, 2), (2, 2)] + \
    [(u0, min(B, ROWS_HALF - u0)) for u0 in range(4, ROWS_HALF, B)]
NBLK = len(BLOCKS)  # 17
TOTAL_MCOLS = ROWS_HALF * NPAIR * Wo  # 206388 flat mask columns

CHUNK_TARGET = 1386   # cut a mask chunk once it reaches this many columns
CHUNK_CAP = 1890      # max chunk columns (target + largest fused mm - 1)
MASK_LEAD = 8         # prefetch chunks ~8 h-steps before first use


def _tap_support(t):
    kh = t // K
    return (0, 0) if kh == 0 else ((1, 3) if kh == 1 else (2, 6))


def _block_mms(u0, nb):
    """Static list of fused stage-2 matmuls for one block.

    Returns ([(t, g, rlo, rhi, col_off)], total_cols): tap t's Z row g
    serves output rows rlo..rhi of the block; mask cols at col_off
    (relative to the block's segment of the flat mask stream).
    """
    out = []
    off = 0
    for t in range(1, NT):
        d0, d1 = _tap_support(t)
        for g in range(u0 + d0, u0 + nb - 1 + d1 + 1):
            rlo = max(u0, g - d1)
            rhi = min(u0 + nb - 1, g - d0)
            if rlo > rhi:
                continue
            out.append((t, g, rlo, rhi, off))
            off += (rhi - rlo + 1) * Wo
    return out, off


def _fire_h(k):
    u0, nb = BLOCKS[k]
    return min(u0 + nb - 1 + DYMAX - 1, SLAB - 1)


def _chunk_plan():
    """Cut the flat mask stream into chunks on fused-mm boundaries.

    Returns (chunks, mm_chunk): chunks = [(gstart, ncols, first_need_h)],
    mm_chunk maps (block k, mm index) -> (chunk_id, local_off).
    """
    chunks = []
    mm_chunk = {}
    goff = 0
    cur_start, cur_len, cur_need = 0, 0, None
    for k, (u0, nb) in enumerate(BLOCKS):
        mms, total = _block_mms(u0, nb)
        for mi, (t, g, rlo, rhi, off) in enumerate(mms):
            n = (rhi - rlo + 1) * Wo
            if cur_need is None:
                cur_need = _fire_h(k)
            mm_chunk[(k, mi)] = (len(chunks), goff + off - cur_start)
            cur_len += n
            if cur_len >= CHUNK_TARGET:
                chunks.append((cur_start, cur_len, cur_need))
                cur_start += cur_len
                cur_len, cur_need = 0, None
        goff += total
    if cur_len:
        chunks.append((cur_start, cur_len, cur_need))
    assert cur_start + cur_len == TOTAL_MCOLS
    assert max(c[1] for c in chunks) <= CHUNK_CAP
    return chunks, mm_chunk


CHUNKS, MM_CHUNK = _chunk_plan()
NCHUNK = len(CHUNKS)


def _interp_bilinear(r, out_h, out_w):
    in_h, in_w = r.shape

    def src(n_out, n_in):
        s = (np.arange(n_out, dtype=np.float32) + 0.5) * (n_in / n_out) - 0.5
        return np.clip(s, 0.0, n_in - 1.0)

    sy = src(out_h, in_h)
    sx = src(out_w, in_w)
    y0 = np.floor(sy).astype(np.int32)
    x0 = np.floor(sx).astype(np.int32)
    y1 = np.minimum(y0 + 1, in_h - 1)
    x1 = np.minimum(x0 + 1, in_w - 1)
    wy = (sy - y0)[:, None]
    wx = (sx - x0)[None, :]
    return (r[y0[:, None], x0[None, :]] * (1 - wy) * (1 - wx)
            + r[y0[:, None], x1[None, :]] * (1 - wy) * wx
            + r[y1[:, None], x0[None, :]] * wy * (1 - wx)
            + r[y1[:, None], x1[None, :]] * wy * wx)


def _build_mask_arrays(rates):
    """Wr[k, d, r, s], Wc[k, d, r, s] float32 with OOB zeroing."""
    rate = _interp_bilinear(rates[0, 0].astype(np.float32), Ho, Wo)
    Wr = np.zeros((K, DYMAX, Ho, Wo), np.float32)
    Wc = np.zeros((K, DYMAX, Ho, Wo), np.float32)
    rr = np.arange(Ho)[:, None]
    ss = np.arange(Wo)[None, :]
    for k in range(K):
        u = k * rate
        f = np.floor(u).astype(np.int32)
        w = u - f
        for d in range(DYMAX):
            v = (f == d) * (1 - w) + (f + 1 == d) * w
            Wr[k, d] = v * (rr + d < H)
            Wc[k, d] = v * (ss + d < W)
    return Wr, Wc


def _build_row_masks(rates, r0):
    """[ROWS_HALF, W, NPAIR*Wo] f32 banded per-row mask tiles (rows r0+...)."""
    Wr, Wc = _build_mask_arrays(rates)
    out = np.zeros((ROWS_HALF, W, NPAIR * Wo), np.float32)
    s = np.arange(Wo)
    for i, (t, kh, kw, dy) in enumerate(PAIRS):
        for dx in range(DYMAX):
            q = s + dx
            valid = q < W
            sv = s[valid]
            M = (Wr[kh, dy, r0:r0 + ROWS_HALF, :][:, sv]
                 * Wc[kw, dx, r0:r0 + ROWS_HALF, :][:, sv])
            rl = np.arange(ROWS_HALF)[:, None]
            out[rl, q[valid][None, :], (i * Wo + sv)[None, :]] = M
    return out


def _pack_flat_masks(row_masks):
    """[W, TOTAL_MCOLS] bf16: flat fused-matmul rhs stream."""
    out = np.zeros((W, TOTAL_MCOLS), np.float32)
    goff = 0
    for k, (u0, nb) in enumerate(BLOCKS):
        mms, total = _block_mms(u0, nb)
        for (t, g, rlo, rhi, off) in mms:
            for j, r in enumerate(range(rlo, rhi + 1)):
                i = PAIR_IDX[(t, g - r)]
                dst = goff + off + j * Wo
                out[:, dst:dst + Wo] = row_masks[r][:, i * Wo:(i + 1) * Wo]
        goff += total
    assert goff == TOTAL_MCOLS
    return out.astype(ml_dtypes.bfloat16)


def build_nc(repeat=1):
    """Build the SPMD program (same for every core)."""
    nc = bacc.Bacc("TRN2", target_bir_lowering=False, debug=False, num_devices=8)
    bf16 = mybir.dt.bfloat16
    f32 = mybir.dt.float32

    xh_d = nc.dram_tensor("xh", [2, C, XHEAD, W], bf16, kind="ExternalInput")
    xm_d = nc.dram_tensor("xm", [2, C, SLAB - XHEAD, W], bf16,
                          kind="ExternalInput")
    w_d = nc.dram_tensor("wpack", [C, WPACK], bf16, kind="ExternalInput")
    b_d = nc.dram_tensor("bias", [O, 1], f32, kind="ExternalInput")
    m_d = nc.dram_tensor("masks", [W, TOTAL_MCOLS], bf16, kind="ExternalInput")
    o_d = nc.dram_tensor("out", [2, O, ROWS_HALF, Wo], f32, kind="ExternalOutput")

    # chunk prefetch schedule: chunk j issues at first_need_h - MASK_LEAD
    issue_at = {}
    for j, (gstart, ncols, need_h) in enumerate(CHUNKS):
        issue_at.setdefault(max(need_h - MASK_LEAD, 0), []).append(j)

    with tile.TileContext(nc) as tc:
        with (
            tc.tile_pool(name="xh", bufs=4) as xhp,
            tc.tile_pool(name="xm", bufs=2) as xmp,
            tc.tile_pool(name="wp", bufs=1) as wp,
            tc.tile_pool(name="zp", bufs=22) as zp,
            tc.tile_pool(name="mp", bufs=12) as mp,
            tc.tile_pool(name="op", bufs=4) as op,
            tc.tile_pool(name="ps1", bufs=3, space="PSUM") as ps1,
            tc.tile_pool(name="acc", bufs=5, space="PSUM") as accp,
        ):
            wt = wp.tile([C, WPACK], bf16, tag="w")
            nc.sync.dma_start(out=wt[:, :], in_=w_d[:, :])
            bt = wp.tile([O, 1], f32, tag="b")
            nc.sync.dma_start(out=bt[:, :], in_=b_d[:, :])

            def body(it):
                # x on the Activation HWDGE queue; masks own the SP queue.
                xhs, xms = [], []
                for img in range(2):
                    xh = xhp.tile([C, XHEAD, W], bf16, tag=f"xh{img}")
                    nc.scalar.dma_start(out=xh[:, :, :], in_=xh_d[img])
                    xhs.append(xh)

                mts = {}

                def mask_dma(j):
                    gstart, ncols, _ = CHUNKS[j]
                    mt = mp.tile([W, CHUNK_CAP], bf16, tag="m", name=f"mc{j}")
                    nc.sync.dma_start(out=mt[:, :ncols],
                                      in_=m_d[:, gstart:gstart + ncols])
                    mts[j] = mt

                for j in issue_at.get(0, []):
                    mask_dma(j)
                for img in range(2):
                    xm = xmp.tile([C, SLAB - XHEAD, W], bf16, tag=f"xm{img}")
                    nc.scalar.dma_start(out=xm[:, :16, :],
                                        in_=xm_d[img][:, :16, :])
                    nc.scalar.dma_start(out=xm[:, 16:, :],
                                        in_=xm_d[img][:, 16:, :])
                    xms.append(xm)

                def xrow(img, h):
                    if h < XHEAD:
                        return xhs[img][:, h, :]
                    return xms[img][:, h - XHEAD, :]

                def xrows(img, r0_, r1_):  # rows r0_..r1_-1, cols :Wo
                    if r1_ <= XHEAD:
                        return xhs[img][:, r0_:r1_, :Wo]
                    return xms[img][:, r0_ - XHEAD:r1_ - XHEAD, :Wo]

                zrows = [{}, {}]  # img -> h -> tile

                # stage-1 weight chunks by h validity:
                #   h<=62: taps 1-8, 63<=h<=65: taps 3-8, 66<=h<=68: taps 6-8
                def stage1(h, img):
                    zt = zp.tile([W, WPACK - O], bf16, tag="z")
                    if h <= ROWS_HALF - 1:
                        chunks = [(0, 512), (512, 512)]
                    elif h <= ROWS_HALF + 2:
                        chunks = [(256, 384), (640, 384)]
                    else:
                        chunks = [(640, 384)]
                    for ci, (z0, width) in enumerate(chunks):
                        p1 = ps1.tile([W, 512], f32, tag="s1")
                        nc.tensor.matmul(
                            p1[:, :width],
                            xrow(img, h),
                            wt[:, O + z0:O + z0 + width],
                            start=True, stop=True,
                        )
                        eng = nc.vector.tensor_copy if (ci + h) % 2 == 0 \
                            else nc.scalar.copy
                        eng(zt[:, z0:z0 + width], p1[:, :width])
                    zrows[img][h] = zt

                def fire_block(k):
                    u0, nb = BLOCKS[k]
                    mms, total = _block_mms(u0, nb)
                    for img in range(2):
                        acc = accp.tile([O, B * Wo], f32, tag="acc")
                        # start=True pending-zeroes the whole 2KB PSUM bank
                        # (zero-region granularity): one fused identity matmul
                        # opens the block's accumulation; everything after
                        # accumulates onto pending-zero.
                        nc.tensor.matmul(
                            acc[:, :nb * Wo],
                            wt[:, :O],
                            xrows(img, u0, u0 + nb),
                            start=True, stop=False,
                        )
                        for mi, (t, g, rlo, rhi, off) in enumerate(mms):
                            n = (rhi - rlo + 1) * Wo
                            cid, loff = MM_CHUNK[(k, mi)]
                            nc.tensor.matmul(
                                acc[:, (rlo - u0) * Wo:(rhi + 1 - u0) * Wo],
                                zrows[img][g][:, ts(t - 1, O)],
                                mts[cid][:, loff:loff + n],
                                start=False, stop=(mi == len(mms) - 1),
                            )
                        ost = op.tile([O, B, Wo], f32, tag="o")
                        nc.scalar.activation(
                            ost[:, :nb, :], acc[:, :nb * Wo],
                            mybir.ActivationFunctionType.Identity,
                            bias=bt[:, :], scale=1.0,
                        )
                        # output DMA on the Activation HWDGE queue
                        nc.scalar.dma_start(
                            out=o_d[img][:, u0:u0 + nb, :],
                            in_=ost[:, :nb, :])

                fire_at = {_fire_h(k): k for k in range(NBLK)}
                for h in range(SLAB):
                    for j in issue_at.get(h, []) if h > 0 else []:
                        mask_dma(j)
                    for img in range(2):
                        stage1(h, img)
                    if h in fire_at:
                        fire_block(fire_at[h])

            if repeat == 1:
                body(0)
            else:
                with tc.For_i(0, repeat, 1):
                    body(0)

    nc.compile()
    return nc


def _prep_core_inputs(inputs, weight, rates, bias):
    """Returns list of 8 in_maps (host-side shard + mask precompute)."""
    x = np.asarray(inputs)
    wgt = np.asarray(weight)
    b = np.asarray(bias)

    # wpack[c, t*O + o] = weight[o, c, kh, kw],  t = kh*K + kw
    wpack = np.transpose(wgt.reshape(O, C, NT), (1, 2, 0)).reshape(C, NT * O)
    wpack = np.ascontiguousarray(wpack).astype(ml_dtypes.bfloat16)
    b2 = np.ascontiguousarray(b.reshape(O, 1)).astype(np.float32)

    masks_by_half = [
        _pack_flat_masks(_build_row_masks(np.asarray(rates), 0)),
        _pack_flat_masks(_build_row_masks(np.asarray(rates), ROWS_HALF)),
    ]

    in_maps = []
    for k in range(8):
        a = 2 * (k // 2)
        half = k % 2
        r0 = ROWS_HALF * half
        slab = np.zeros((2, C, SLAB, W), np.float32)
        avail = min(SLAB, H - r0)
        slab[0, :, :avail, :] = x[a, :, r0:r0 + avail, :]
        slab[1, :, :avail, :] = x[a + 1, :, r0:r0 + avail, :]
        slab16 = slab.astype(ml_dtypes.bfloat16)
        in_maps.append({
            "xh": np.ascontiguousarray(slab16[:, :, :XHEAD, :]),
            "xm": np.ascontiguousarray(slab16[:, :, XHEAD:, :]),
            "wpack": wpack,
            "bias": b2,
            "masks": masks_by_half[half],
        })
    return in_maps


_NC_CACHE = {}


def _get_nc(repeat=1):
    if repeat not in _NC_CACHE:
        _NC_CACHE[repeat] = build_nc(repeat)
    return _NC_CACHE[repeat]


def kernel(inputs, weight, rates, bias):
    nc = _get_nc(1)
    in_maps = _prep_core_inputs(inputs, weight, rates, bias)
    res = run_bass_kernel_spmd(nc, in_maps, core_ids=list(range(8)))
    out = np.zeros((NIMG, O, Ho, Wo), np.float32)
    for k in range(8):
        a = 2 * (k // 2)
        half = k % 2
        r0 = ROWS_HALF * half
        o = res.results[k]["out"]
        out[a, :, r0:r0 + ROWS_HALF, :] = o[0]
        out[a + 1, :, r0:r0 + ROWS_HALF, :] = o[1]
    return out
